# revision 19
# baseline (speedup 1.0000x reference)
"""AdaLN (DiT-style) transformer block on 8 Trainium2 NeuronCores.

Data-parallel over batch: core b computes batch element b end-to-end
(B == n_cores == 8), no collectives.

The AdaLN conditioning is folded into the weights HOST-side (exact):
  c = silu(cond) @ cond_w.T + cond_b -> s1 b1 g1 s2 b2 g2  (per batch)
  (1+s) scales W columns; b folds into biases; k-bias dropped (softmax
  invariance); v-bias folds into proj bias (softmax rows sum to 1);
  g1/g2 fold into proj/w2 rows.
Device kernel: LN -> qkv -> attention -> proj+res -> LN -> mlp1 ->
silu -> mlp2 + res.  Matmul datapath bf16 (fp32 PSUM/LN/denoms/res).

Schedule keeps the PE continuously busy (TRN2 drops the PE clock
2.4->1.2 GHz for 3us after any idle gap):
 - identities DMA'd from DRAM (no gpsimd preamble), warm matmuls
   cover the input DMA window;
 - attention processes HEAD PAIRS fused at j-tile granularity (the
   pair shares qkT tiles); the next pair's qkT emission and the
   deferred softmax-normalize fill the PE/vector slack; exp on the
   Scalar engine is the phase's true floor (~50us);
 - proj/LN2/mlp1/mlp2 interleave so the PE never drains at phase
   boundaries.

Fixed problem shape: x [8, 1024, 384], cond [8, 384], H=6 heads, hd=64.
"""
import sys

if '/opt/trn_rl_repo' not in sys.path:
    sys.path.insert(0, '/opt/trn_rl_repo')

import ml_dtypes
import numpy as np

import concourse.bacc as bacc
import concourse.tile as tile
from concourse import mybir
from concourse.bass_utils import run_bass_kernel_spmd

B, L, D, H = 8, 1024, 384, 6
HD = D // H                  # 64
DQ = 3 * D                   # 1152
DM = 4 * D                   # 1536
KD = D // 128                # 3 k-tiles over D
IT = L // 128                # 8 i-tiles over L
IC = L // 512                # 2 512-chunks over L
SCALE = HD ** -0.5
EPS = 1e-5

f32 = mybir.dt.float32
bf16 = mybir.dt.bfloat16
ACTF = mybir.ActivationFunctionType
ALU = mybir.AluOpType
BF = ml_dtypes.bfloat16

_cache = {}


def build(use_m2b):
    nc = bacc.Bacc()

    identd = nc.declare_dram_parameter("identd", [128, 128], f32, isOutput=False)
    identbd = nc.declare_dram_parameter("identbd", [128, 128], bf16, isOutput=False)
    xb = nc.declare_dram_parameter("xb", [L, D], f32, isOutput=False)
    qkvw = nc.declare_dram_parameter("qkvw", [D, DQ], bf16, isOutput=False)
    qcols = nc.declare_dram_parameter("qcols", [D], f32, isOutput=False)
    projw = nc.declare_dram_parameter("projw", [D, D], bf16, isOutput=False)
    projb = nc.declare_dram_parameter("projb", [D], bf16, isOutput=False)
    w1 = nc.declare_dram_parameter("w1", [D, DM], bf16, isOutput=False)
    m1b = nc.declare_dram_parameter("m1b", [DM], f32, isOutput=False)
    w2 = nc.declare_dram_parameter("w2", [DM, D], bf16, isOutput=False)
    if use_m2b:
        m2b = nc.declare_dram_parameter("m2b", [D], bf16, isOutput=False)
    out = nc.declare_dram_parameter("out", [L, D], f32, isOutput=True)

    with tile.TileContext(nc) as tc:
        from contextlib import ExitStack
        ctx = ExitStack()
        with ctx:
            persist = ctx.enter_context(tc.tile_pool(name="persist", bufs=1))
            sb = ctx.enter_context(tc.tile_pool(name="small", bufs=4))
            hpool = ctx.enter_context(tc.tile_pool(name="hpool", bufs=3))
            exps = ctx.enter_context(tc.tile_pool(name="exps", bufs=6))
            dramp = ctx.enter_context(tc.tile_pool(name="dramp", bufs=1, space="DRAM"))

            # ------------- DMAs (priority order) -------------
            ident = persist.tile([128, 128], f32, tag="ident")
            nc.sync.dma_start(out=ident, in_=identd[:, :])
            identb = persist.tile([128, 128], bf16, tag="identb")
            nc.sync.dma_start(out=identb, in_=identbd[:, :])
            xt2 = [persist.tile([128, 2 * D], f32, name=f"x{j}", tag=f"x{j}")
                   for j in range(IT // 2)]
            for j in range(IT // 2):
                nc.sync.dma_start(
                    out=xt2[j][:, :].rearrange("p (a f) -> p a f", f=D),
                    in_=xb[j * 256:(j + 1) * 256, :].rearrange(
                        "(a p) f -> p a f", p=128))
            xv = lambda i: xt2[i // 2][:, (i % 2) * D:(i % 2 + 1) * D]
            qkvw_sb = [persist.tile([128, DQ], bf16, name=f"qkvw{k}", tag=f"qkvw{k}")
                       for k in range(KD)]
            for k in range(KD):
                nc.sync.dma_start(out=qkvw_sb[k], in_=qkvw[k * 128:(k + 1) * 128, :])
            qcols_sb = persist.tile([128, KD], f32, tag="qcols")
            nc.sync.dma_start(out=qcols_sb, in_=qcols[:].rearrange("(t p) -> p t", p=128))
            projw_sb = [persist.tile([128, D], bf16, name=f"projw{k}", tag=f"projw{k}")
                        for k in range(KD)]
            for k in range(KD):
                nc.sync.dma_start(out=projw_sb[k], in_=projw[k * 128:(k + 1) * 128, :])
            pbrow = persist.tile([1, D], bf16, tag="pbrow")
            nc.sync.dma_start(out=pbrow, in_=projb[:].rearrange("(o f) -> o f", o=1))
            w1_sb = [persist.tile([128, DM], bf16, name=f"w1_{k}", tag=f"w1_{k}")
                     for k in range(KD)]
            for k in range(KD):
                nc.sync.dma_start(out=w1_sb[k], in_=w1[k * 128:(k + 1) * 128, :])
            m1cols = persist.tile([128, 12], f32, tag="m1cols")
            nc.sync.dma_start(out=m1cols, in_=m1b[:].rearrange("(t p) -> p t", p=128))
            w2_sb = [persist.tile([128, D], bf16, name=f"w2_{k}", tag=f"w2_{k}")
                     for k in range(12)]
            for k in range(12):
                nc.sync.dma_start(out=w2_sb[k], in_=w2[k * 128:(k + 1) * 128, :])
            if use_m2b:
                m2row = persist.tile([1, D], bf16, tag="m2row")
                nc.sync.dma_start(out=m2row, in_=m2b[:].rearrange("(o f) -> o f", o=1))

            # ------------- constants -------------
            eps_t = persist.tile([128, 1], f32, tag="eps")
            nc.vector.memset(eps_t, EPS)
            ones6_f = persist.tile([128, 6, 1], bf16, tag="ones6f")
            nc.vector.memset(ones6_f, 1.0)
            onesb = persist.tile([1, 128], bf16, tag="onesb")
            nc.vector.memset(onesb, 1.0)

            wdram = dramp.tile([1, 1], f32, name="wdram", tag="wdram")

            def pe_warm(n, pool, tg):
                """n dummy bf16 matmuls to hold/raise the PE clock."""
                wp = pool.tile([128, 384], f32, name="warm", tag=tg)
                for w in range(n):
                    nc.tensor.matmul(wp[:, 0:128], identb[:, :], identb[:, :],
                                     start=(w == 0), stop=(w == n - 1))
                ws = sb.tile([1, 1], f32, name="wsink", tag="wsink")
                nc.scalar.copy(ws, wp[0:1, 0:1])
                nc.sync.dma_start(out=wdram, in_=ws)

            def _layernorm(i_idx, xt_i, ln_out):
                """LN over free dim; stats on vector, apply on gpsimd."""
                stats = sb.tile([128, 6], f32, name="ln_stats", tag="ln_stats")
                nc.vector.bn_stats(out=stats, in_=xt_i)
                mv = sb.tile([128, 2], f32, name="ln_mv", tag="ln_mv")
                nc.vector.bn_aggr(out=mv, in_=stats)
                rstd = sb.tile([128, 1], f32, name="ln_rstd", tag="ln_rstd")
                nc.scalar.activation(out=rstd, in_=mv[:, 1:2], func=ACTF.Sqrt,
                                     bias=eps_t, scale=1.0)
                nc.vector.reciprocal_approx_fast(rstd, rstd)
                negmr = sb.tile([128, 1], f32, name="ln_negmr", tag="ln_negmr")
                nc.gpsimd.tensor_scalar(out=negmr, in0=mv[:, 0:1], scalar1=rstd,
                                        scalar2=-1.0, op0=ALU.mult, op1=ALU.mult)
                nc.gpsimd.tensor_scalar(out=ln_out, in0=xt_i, scalar1=rstd,
                                        scalar2=negmr, op0=ALU.mult, op1=ALU.add)

            # ---------------- phase A: LN1 + v + qkT(0,3) ----------------
            with tc.tile_pool(name="attn1", bufs=1) as ap1, \
                 tc.tile_pool(name="attn2", bufs=1) as ap2:
                # lnT as one [128, KD, L] tensor so each i-tile needs ONE
                # PSUM->SBUF copy for all three k chunks.
                lnT = ap1.tile([128, KD, L], bf16, name="lnT", tag="lnT")
                qkT = [ap1.tile([128, L], bf16, name=f"qkT{t}", tag=f"qkT{t}")
                       for t in range(6)]
                vsb = [ap1.tile([128, 6 * (HD + 1)], bf16, name=f"v{j}", tag=f"v{j}")
                       for j in range(IT)]
                attnT = [ap2.tile([128, L], bf16, name=f"attnT{k}", tag=f"attnT{k}")
                         for k in range(KD)]

                with tc.tile_pool(name="ps_a", bufs=2, space="PSUM") as ps_a, \
                     tc.tile_pool(name="ps_tp", bufs=2, space="PSUM") as ps_tp:
                    # warm the PE while the x/qkv DMAs land
                    pe_warm(56, ps_a, "mm")

                    for i in range(IT):
                        ln = hpool.tile([128, D], bf16, name="ln1", tag="h1")
                        _layernorm(i, xv(i), ln)
                        pt3 = ps_tp.tile([128, D], bf16, name="pt3", tag="tp")
                        for k in range(KD):
                            nc.tensor.transpose(pt3[:, k * 128:(k + 1) * 128],
                                                ln[:, k * 128:(k + 1) * 128],
                                                identb[:, :])
                        dst = lnT[:, :, i * 128:(i + 1) * 128]
                        src = pt3[:, :].rearrange("p (k c) -> p k c", c=128)
                        if i % 2 == 0:
                            nc.vector.tensor_copy(dst, src)
                        else:
                            nc.scalar.copy(dst, src)
                        # v(0..1) here; v(2..7) interleave into pair 0
                        # of the attention loop as PE filler.
                        if i < 2:
                            pv = ps_a.tile([128, D], f32, name="pv", tag="mm")
                            for k in range(KD):
                                nc.tensor.matmul(pv[:, :],
                                                 lnT[:, k, i * 128:(i + 1) * 128],
                                                 qkvw_sb[k][:, 768:1152],
                                                 start=(k == 0), stop=(k == KD - 1))
                            vview = vsb[i][:, :].rearrange("p (h c) -> p h c", c=HD + 1)
                            pvview = pv[:, :].rearrange("p (h c) -> p h c", c=HD)
                            nc.scalar.copy(vview[:, :, 0:HD], pvview)
                            nc.gpsimd.tensor_copy(vview[:, :, HD:HD + 1], ones6_f)

                # -------------- phase B: attention (head pairs) --------------
                with tc.tile_pool(name="ps_s", bufs=2, space="PSUM") as ps_s, \
                     tc.tile_pool(name="ps_o0", bufs=2, space="PSUM") as ps_o0, \
                     tc.tile_pool(name="ps_o1", bufs=2, space="PSUM") as ps_o1:

                    def emit_qkT(t, on_scalar=False):
                        # q (t<3) / k (t>=3) head-pair -> qkT[t]; PSUM from
                        # the S ring; copies on vector during attention
                        # (Scalar is exp-bound there), scalar in phase A.
                        pq = ps_s.tile([128, L], f32, name="pss", tag="s")
                        for ic in range(IC):
                            for k in range(KD):
                                nc.tensor.matmul(pq[:, ic * 512:(ic + 1) * 512],
                                                 qkvw_sb[k][:, t * 128:(t + 1) * 128],
                                                 lnT[:, k, ic * 512:(ic + 1) * 512],
                                                 start=(k == 0), stop=(k == KD - 1))
                        for ic in range(IC):
                            dst = qkT[t][:, ic * 512:(ic + 1) * 512]
                            src = pq[:, ic * 512:(ic + 1) * 512]
                            if t < 3:
                                if on_scalar:
                                    nc.scalar.activation(out=dst, in_=src,
                                                         func=ACTF.Identity,
                                                         bias=qcols_sb[:, t:t + 1],
                                                         scale=1.0)
                                else:
                                    nc.vector.tensor_scalar_add(dst, src,
                                                                qcols_sb[:, t:t + 1])
                            elif on_scalar:
                                nc.scalar.copy(dst, src)
                            else:
                                nc.vector.tensor_copy(dst, src)

                    emit_qkT(0, on_scalar=True)
                    emit_qkT(3)

                    po = {}
                    es_t = {}

                    def emit_v(i):
                        pv = ps_s.tile([128, L], f32, name="pss", tag="s")
                        for k in range(KD):
                            nc.tensor.matmul(pv[:, 0:D],
                                             lnT[:, k, i * 128:(i + 1) * 128],
                                             qkvw_sb[k][:, 768:1152],
                                             start=(k == 0), stop=(k == KD - 1))
                        vview = vsb[i][:, :].rearrange("p (h c) -> p h c", c=HD + 1)
                        pvview = pv[:, 0:D].rearrange("p (h c) -> p h c", c=HD)
                        nc.vector.tensor_copy(vview[:, :, 0:HD], pvview)
                        nc.gpsimd.tensor_copy(vview[:, :, HD:HD + 1], ones6_f)

                    def emit_s(h, jt):
                        tq = h // 2
                        ro = (h % 2) * HD
                        pss = ps_s.tile([128, L], f32, name="pss", tag="s")
                        for ic in range(IC):
                            nc.tensor.matmul(pss[:, ic * 512:(ic + 1) * 512],
                                             qkT[3 + tq][ro:ro + HD, jt * 128:(jt + 1) * 128],
                                             qkT[tq][ro:ro + HD, ic * 512:(ic + 1) * 512],
                                             start=True, stop=True)
                        es = exps.tile([128, L], bf16, name="es", tag="expS")
                        nc.scalar.activation(out=es, in_=pss[:, :], func=ACTF.Exp,
                                             scale=SCALE)
                        es_t[(h, jt)] = es

                    def emit_pv(h, jt):
                        po_pool = ps_o0 if h % 2 == 0 else ps_o1
                        if jt == 0:
                            po[h] = [po_pool.tile([HD + 1, 512], f32, name=f"po{ic}",
                                                  tag=f"po{h % 2}")
                                     for ic in range(IC)]
                        for ic in range(IC):
                            nc.tensor.matmul(po[h][ic][:, :],
                                             vsb[jt][:, h * (HD + 1):(h + 1) * (HD + 1)],
                                             es_t[(h, jt)][:, ic * 512:(ic + 1) * 512],
                                             start=(jt == 0), stop=(jt == IT - 1))
                        es_t.pop((h, jt), None)

                    def normalize(h):
                        tq = h // 2
                        ro = (h % 2) * HD
                        for ic in range(IC):
                            dn = sb.tile([1, 512], f32, name="dn", tag="dn", bufs=3)
                            nc.vector.tensor_copy(dn, po[h][ic][HD:HD + 1, :])
                            rcp = sb.tile([1, 512], f32, name="rcp", tag="rcp", bufs=3)
                            nc.vector.reciprocal_approx_fast(rcp, dn)
                            rcpb = sb.tile([HD, 512], f32, name="rcpb", tag="rcpb", bufs=3)
                            nc.gpsimd.partition_broadcast(rcpb, rcp[:1, :])
                            nc.vector.tensor_mul(
                                attnT[tq][ro:ro + HD, ic * 512:(ic + 1) * 512],
                                po[h][ic][0:HD, :], rcpb)
                        del po[h]

                    # fused head-pair loop: S/PV alternate at j-tile
                    # granularity; v-emission (pair 0) and the next pair's
                    # qkT lumps fill the PE slack so the PE never idles
                    # (any idle gap halves the PE clock for 3us).
                    for hp in range(3):
                        h0, h1 = 2 * hp, 2 * hp + 1
                        if hp == 0:
                            emit_s(h0, 0)
                            emit_s(h1, 0)
                        for jt in range(1, IT):
                            emit_s(h0, jt)
                            emit_pv(h0, jt - 1)
                            emit_s(h1, jt)
                            emit_pv(h1, jt - 1)
                            if hp == 0 and jt < IT - 1:
                                emit_v(jt + 1)
                            if hp < 2:
                                if jt == 3:
                                    emit_qkT(hp + 1)        # next pair's q
                                elif jt == 5:
                                    emit_qkT(4 + hp)        # next pair's k
                        if hp < 2:
                            emit_s(h0 + 2, 0)
                        emit_pv(h0, IT - 1)
                        if hp < 2:
                            emit_s(h1 + 2, 0)
                        emit_pv(h1, IT - 1)
                        normalize(h0)
                        normalize(h1)

                # -------------- phase C/D: proj+res+LN2 & MLP --------------
                x1t = [persist.tile([128, D], f32, name=f"x1_{i}", tag=f"x1_{i}")
                       for i in range(IT)]
                with tc.tile_pool(name="mlp1", bufs=1) as mp1, \
                     tc.tile_pool(name="ps_c", bufs=2, space="PSUM") as ps_c, \
                     tc.tile_pool(name="ps_c2", bufs=2, space="PSUM") as ps_c2, \
                     tc.tile_pool(name="ps_m", bufs=4, space="PSUM") as ps_m:
                    h2T = mp1.tile([128, KD, L], bf16, name="h2T", tag="h2T")
                    siluT = [mp1.tile([128, L], bf16, name=f"siluT{t}", tag=f"siluT{t}")
                             for t in range(12)]

                    # keep the PE clocked while head-5 normalize drains
                    pe_warm(16, ps_c, "mm")

                    def emit_proj(i):
                        py = ps_c.tile([128, D], f32, name="py", tag="mm")
                        for k in range(KD):
                            nc.tensor.matmul(py[:, :],
                                             attnT[k][:, i * 128:(i + 1) * 128],
                                             projw_sb[k][:, :],
                                             start=(k == 0), stop=False)
                        nc.tensor.matmul(py[:, :], onesb[:1, :], pbrow[:1, :],
                                         start=False, stop=True)
                        nc.vector.tensor_add(x1t[i], xv(i), py[:, :])

                    def emit_ln2(i):
                        ln = hpool.tile([128, D], bf16, name="ln2", tag="h2")
                        _layernorm(i, x1t[i], ln)
                        pt3 = ps_c2.tile([128, D], bf16, name="pt2", tag="tp")
                        for k in range(KD):
                            nc.tensor.transpose(pt3[:, k * 128:(k + 1) * 128],
                                                ln[:, k * 128:(k + 1) * 128],
                                                identb[:, :])
                        dst = h2T[:, :, i * 128:(i + 1) * 128]
                        src = pt3[:, :].rearrange("p (k c) -> p k c", c=128)
                        nc.scalar.copy(dst, src)

                    def emit_mlp1(ic, t):
                        pa = ps_m.tile([128, 512], f32, name="pa", tag="m")
                        for k in range(KD):
                            nc.tensor.matmul(pa[:, :],
                                             w1_sb[k][:, t * 128:(t + 1) * 128],
                                             h2T[:, k, ic * 512:(ic + 1) * 512],
                                             start=(k == 0), stop=(k == KD - 1))
                        nc.scalar.activation(out=siluT[t][:, ic * 512:(ic + 1) * 512],
                                             in_=pa[:, :], func=ACTF.Silu,
                                             bias=m1cols[:, t:t + 1], scale=1.0)

                    def emit_mlp2(i):
                        py = ps_m.tile([128, D], f32, name="py2", tag="m")
                        for k in range(12):
                            nc.tensor.matmul(py[:, :],
                                             siluT[k][:, i * 128:(i + 1) * 128],
                                             w2_sb[k][:, :],
                                             start=(k == 0),
                                             stop=(k == 11 and not use_m2b))
                        if use_m2b:
                            nc.tensor.matmul(py[:, :], onesb[:1, :], m2row[:1, :],
                                             start=False, stop=True)
                        ot = hpool.tile([128, D], f32, name="ot", tag="ot")
                        nc.vector.tensor_add(ot, x1t[i], py[:, :])
                        nc.sync.dma_start(out=out[i * 128:(i + 1) * 128, :], in_=ot)

                    # proj leads LN2 by 3 i-tiles; mlp1(ic=0) starts once
                    # h2T covers i 0..3; mlp2 interleaves into mlp1(ic=1).
                    emit_proj(0)
                    emit_proj(1)
                    emit_proj(2)
                    for i in range(3, IT):
                        emit_proj(i)
                        emit_ln2(i - 3)
                    for i in range(IT - 3, IT - 1):
                        emit_ln2(i)
                    for t in range(4):
                        emit_mlp1(0, t)
                    emit_ln2(IT - 1)
                    for t in range(4, 12):
                        emit_mlp1(0, t)
                    for t in range(6):
                        emit_mlp1(1, t)
                    for i in range(4):
                        emit_mlp2(i)
                    for t in range(6, 12):
                        emit_mlp1(1, t)
                    for i in range(4, IT):
                        emit_mlp2(i)

    nc.compile()
    return nc


def _get_nc(use_m2b):
    if use_m2b not in _cache:
        _cache[use_m2b] = build(use_m2b)
    return _cache[use_m2b]


def _silu(x):
    return x / (1.0 + np.exp(-x))


_IDENT_F32 = np.eye(128, dtype=np.float32)
_IDENT_BF16 = np.eye(128, dtype=np.float32).astype(BF)


def fold_params(cond, qkv_w, qkv_b, proj_w, proj_b, mlp_w1, mlp_b1, mlp_w2,
                mlp_b2, cond_w, cond_b):
    """Fold the AdaLN conditioning into per-batch weights (fp32 host math)."""
    c = _silu(cond) @ cond_w.T + cond_b          # [B, 6D]
    s1, b1, g1, s2, b2, g2 = np.split(c, 6, axis=-1)
    outs = []
    for b in range(cond.shape[0]):
        qkv_eff = qkv_w * (1.0 + s1[b])[None, :]           # [3D, D]
        qkv_bias = qkv_b + qkv_w @ b1[b]                   # [3D]
        bq = qkv_bias[0:D]
        bv = qkv_bias[2 * D:3 * D]
        projw_eff = proj_w * g1[b][:, None]                # [D, D] rows=out
        projb_eff = g1[b] * (proj_b + bv @ proj_w.T)       # [D]
        w1_eff = mlp_w1 * (1.0 + s2[b])[None, :]           # [4D, D]
        m1b_eff = mlp_b1 + mlp_w1 @ b2[b]                  # [4D]
        w2_eff = mlp_w2 * g2[b][:, None]                   # [D, 4D]
        m2b_eff = g2[b] * mlp_b2                           # [D]
        outs.append({
            "identd": _IDENT_F32,
            "identbd": _IDENT_BF16,
            "qkvw": np.ascontiguousarray(qkv_eff.T.astype(BF)),
            "qcols": np.ascontiguousarray(bq.astype(np.float32)),
            "projw": np.ascontiguousarray(projw_eff.T.astype(BF)),
            "projb": np.ascontiguousarray(projb_eff.astype(BF)),
            "w1": np.ascontiguousarray(w1_eff.T.astype(BF)),
            "m1b": np.ascontiguousarray(m1b_eff.astype(np.float32)),
            "w2": np.ascontiguousarray(w2_eff.T.astype(BF)),
            "m2b": np.ascontiguousarray(m2b_eff.astype(BF)),
        })
    return outs


def kernel(x, cond, qkv_w, qkv_b, proj_w, proj_b, mlp_w1, mlp_b1, mlp_w2, mlp_b2,
           cond_w, cond_b, num_heads):
    x = np.asarray(x, np.float32)
    cond = np.asarray(cond, np.float32)
    qkv_w = np.asarray(qkv_w, np.float32)
    qkv_b = np.asarray(qkv_b, np.float32)
    proj_w = np.asarray(proj_w, np.float32)
    proj_b = np.asarray(proj_b, np.float32)
    mlp_w1 = np.asarray(mlp_w1, np.float32)
    mlp_b1 = np.asarray(mlp_b1, np.float32)
    mlp_w2 = np.asarray(mlp_w2, np.float32)
    mlp_b2 = np.asarray(mlp_b2, np.float32)
    cond_w = np.asarray(cond_w, np.float32)
    cond_b = np.asarray(cond_b, np.float32)
    assert int(num_heads) == H and x.shape == (B, L, D)

    folded = fold_params(cond, qkv_w, qkv_b, proj_w, proj_b, mlp_w1, mlp_b1,
                         mlp_w2, mlp_b2, cond_w, cond_b)
    use_m2b = bool(any(np.any(f["m2b"]) for f in folded))
    nc = _get_nc(use_m2b)

    in_maps = []
    for b in range(B):
        m = dict(folded[b], xb=np.ascontiguousarray(x[b]))
        if not use_m2b:
            m.pop("m2b")
        in_maps.append(m)
    res = run_bass_kernel_spmd(nc, in_maps, list(range(B)))
    return np.stack([res.results[b]["out"] for b in range(B)], axis=0)
